# revision 4
# baseline (speedup 1.0000x reference)
"""Trainium2 Bass kernel for the D4RT loss (segment_reduce).

Batch-parallel over 8 NeuronCores (one batch element per core). Per core,
one NEFF with two phases:
  Phase A: per-group depth sums/counts via nibble one-hot matmuls on the
           TensorEngine (contraction over 128 points per column).
  Epilogue: 64-entry mean-depth reciprocal tables computed on-chip, bounced
           through DRAM to broadcast across all 128 partitions.
  Phase B: streaming elementwise losses; per-point table gather is a 64-wide
           one-hot multiply-reduce on the VectorEngine.
Host combines per-core scalar partials.
"""
import sys, os

for _p in ("/opt/trn_rl_repo", os.path.expanduser("~/.axon_site/_ro/trn_rl_repo")):
    if os.path.isdir(_p) and _p not in sys.path:
        sys.path.insert(0, _p)

import numpy as np
import concourse.bacc as bacc
import concourse.mybir as mybir
from concourse.tile import TileContext
from concourse.bass_utils import run_bass_kernel_spmd

dt = mybir.dt
Alu = mybir.AluOpType
Act = mybir.ActivationFunctionType
AX = mybir.AxisListType.X

B, N, G = 8, 262144, 64
P = 128               # SBUF partitions
FT = N // P           # 2048 points per partition per core
FA = 512              # phase tile size (points per partition per tile)
NT = FT // FA         # 4 tiles
FG = 32               # gather sub-chunk size
EPS = 1e-6

_COMPILED = {}


def v3t(t, c, i):
    # [N, c] dram -> tile i view [P, FA*c]
    return t.ap().rearrange("(p t f) c -> t p (f c)", p=P, t=NT)[i]


def v1t(t, i):
    return t.ap().rearrange("(p t f) -> t p f", p=P, t=NT)[i]


def _build():
    nc = bacc.Bacc("TRN2", target_bir_lowering=False, debug=False, num_devices=8)

    def din(name, shape):
        return nc.dram_tensor(name, shape, dt.float32, kind="ExternalInput")

    pp = din("pred_points", [N, 3])
    tp = din("target_points", [N, 3])
    p2 = din("pred_2d", [N, 2])
    t2 = din("target_2d", [N, 2])
    pv = din("pred_vis", [N, 1])
    tv = din("target_vis", [N])
    pd = din("pred_disp", [N, 3])
    td = din("target_disp", [N, 3])
    pn = din("pred_normal", [N, 3])
    tn = din("target_normal", [N, 3])
    cf = din("confidence", [N, 1])
    mk = nc.dram_tensor("mask", [N], dt.int32, kind="ExternalInput")
    gr = nc.dram_tensor("groups", [N], dt.int32, kind="ExternalInput")

    stats_out = nc.dram_tensor("stats", [P, 8], dt.float32, kind="ExternalOutput")
    gstats_out = nc.dram_tensor("gstats", [8, 24], dt.float32, kind="ExternalOutput")
    scratch = nc.dram_tensor("tbl_scratch", [2, G], dt.float32)

    with TileContext(nc) as tc:
        with tc.tile_pool(name="res", bufs=1) as rp:
            P_res = rp.tile([P, FT * 3], dt.float32, tag="Pres")
            T_res = rp.tile([P, FT * 3], dt.float32, tag="Tres")
            w_res = rp.tile([P, FT], dt.float32, tag="wres")
            gmx_res = rp.tile([P, FT], dt.int32, tag="gmxres")
            tblrep = rp.tile([P, 2 * G], dt.float32, tag="tblrep")
            iotas = rp.tile([P, 80], dt.int32, tag="iotas")
            stats_t = rp.tile([P, 8], dt.float32, tag="stats")
            gs_sb = rp.tile([8, 24], dt.float32, tag="gs")

            iota_hi = iotas[:, 0:8]
            iota_lo = iotas[:, 8:16]
            iota64 = iotas[:, 16:80]

            nc.sync.dma_start(out=P_res[:, :],
                              in_=pp.ap().rearrange("(p f) c -> p (f c)", p=P))
            nc.sync.dma_start(out=T_res[:, :],
                              in_=tp.ap().rearrange("(p f) c -> p (f c)", p=P))
            nc.gpsimd.iota(iota_hi, pattern=[[1, 8]], base=8, channel_multiplier=0)
            nc.gpsimd.iota(iota_lo, pattern=[[1, 8]], base=0, channel_multiplier=0)
            nc.gpsimd.iota(iota64, pattern=[[1, G]], base=G, channel_multiplier=0)
            nc.vector.memset(stats_t[:, :], 0.0)

            with tc.tile_pool(name="gm", bufs=1) as gmp:
                g_t = gmp.tile([P, FT], dt.int32)
                m_t = gmp.tile([P, FT], dt.int32)
                nc.sync.dma_start(out=g_t[:, :],
                                  in_=gr.ap().rearrange("(p f) -> p f", p=P))
                nc.sync.dma_start(out=m_t[:, :],
                                  in_=mk.ap().rearrange("(p f) -> p f", p=P))
                # gmx = groups + 64*mask (valid -> [64,128), invalid -> [0,64))
                nc.vector.scalar_tensor_tensor(
                    out=gmx_res[:, :], in0=m_t[:, :], scalar=64.0, in1=g_t[:, :],
                    op0=Alu.mult, op1=Alu.add)
                nc.vector.tensor_copy(w_res[:, :], m_t[:, :])  # i32 -> f32

                # ================= Phase A: group stats =================
                with (
                    tc.tile_pool(name="pa", bufs=1) as pa,
                    tc.tile_pool(name="ps", bufs=2, space="PSUM") as psp,
                ):
                    for i in range(NT):
                        fs = slice(i * FA, (i + 1) * FA)
                        hi_t = pa.tile([P, FA], dt.int32, tag="hi")
                        lo_t = pa.tile([P, FA], dt.int32, tag="lo")
                        nc.vector.tensor_scalar(out=hi_t[:, :], in0=gmx_res[:, fs],
                                                scalar1=3, scalar2=None,
                                                op0=Alu.logical_shift_right)
                        nc.vector.tensor_scalar(out=lo_t[:, :], in0=gmx_res[:, fs],
                                                scalar1=7, scalar2=None,
                                                op0=Alu.bitwise_and)
                        ohhi = pa.tile([P, FA * 8], dt.float32, tag="ohhi")
                        rhs = pa.tile([P, FA * 24], dt.float32, tag="rhs")
                        ohhi3 = ohhi[:, :].rearrange("p (f r) -> p f r", r=8)
                        rhs3 = rhs[:, :].rearrange("p (f k) -> p f k", k=24)
                        hi_b = hi_t[:, :].unsqueeze(2).broadcast_to([P, FA, 8])
                        lo_b = lo_t[:, :].unsqueeze(2).broadcast_to([P, FA, 8])
                        ihi_b = iota_hi.unsqueeze(1).broadcast_to([P, FA, 8])
                        ilo_b = iota_lo.unsqueeze(1).broadcast_to([P, FA, 8])
                        nc.vector.tensor_tensor(out=ohhi3, in0=hi_b, in1=ihi_b,
                                                op=Alu.is_equal)
                        nc.vector.tensor_tensor(out=rhs3[:, :, 0:8], in0=lo_b,
                                                in1=ilo_b, op=Alu.is_equal)
                        Pv = P_res[:, :].rearrange("p (f c) -> p f c", c=3)
                        Tv = T_res[:, :].rearrange("p (f c) -> p f c", c=3)
                        zp_b = Pv[:, fs, 2].unsqueeze(2).broadcast_to([P, FA, 8])
                        zt_b = Tv[:, fs, 2].unsqueeze(2).broadcast_to([P, FA, 8])
                        nc.vector.tensor_tensor(out=rhs3[:, :, 8:16],
                                                in0=rhs3[:, :, 0:8], in1=zp_b,
                                                op=Alu.mult)
                        nc.vector.tensor_tensor(out=rhs3[:, :, 16:24],
                                                in0=rhs3[:, :, 0:8], in1=zt_b,
                                                op=Alu.mult)
                        acc = psp.tile([8, 24], dt.float32, tag="acc")
                        for f in range(FA):
                            nc.tensor.matmul(acc[:, :], ohhi3[:, f, :], rhs3[:, f, :],
                                             start=(f == 0), stop=(f == FA - 1))
                        if i == 0:
                            nc.vector.tensor_copy(gs_sb[:, :], acc[:, :])
                        else:
                            nc.vector.tensor_tensor(out=gs_sb[:, :], in0=gs_sb[:, :],
                                                    in1=acc[:, :], op=Alu.add)

            nc.sync.dma_start(out=gstats_out[:, :], in_=gs_sb[:, :])

            # ================= Epilogue: tables =================
            with tc.tile_pool(name="ep", bufs=1) as ep:
                cnt = gs_sb[:, 0:8]
                cntm = ep.tile([8, 8], dt.float32, tag="cntm")
                nc.vector.tensor_scalar(out=cntm[:, :], in0=cnt, scalar1=1.0,
                                        scalar2=None, op0=Alu.max)
                nc.vector.reciprocal(cntm[:, :], cntm[:, :])
                z0 = ep.tile([8, 8], dt.float32, tag="z0")
                nc.vector.tensor_scalar(out=z0[:, :], in0=cnt, scalar1=0.0,
                                        scalar2=None, op0=Alu.is_gt)
                z1 = ep.tile([8, 8], dt.float32, tag="z1")  # 1 - z0
                nc.vector.tensor_scalar(out=z1[:, :], in0=z0[:, :], scalar1=-1.0,
                                        scalar2=1.0, op0=Alu.mult, op1=Alu.add)
                tbl_sb = ep.tile([8, 16], dt.float32, tag="tbl")
                mean = ep.tile([8, 8], dt.float32, tag="mean")
                for c, col in ((0, slice(8, 16)), (1, slice(16, 24))):
                    nc.vector.tensor_tensor(out=mean[:, :], in0=gs_sb[:, col],
                                            in1=cntm[:, :], op=Alu.mult)
                    nc.vector.tensor_tensor(out=mean[:, :], in0=mean[:, :],
                                            in1=z0[:, :], op=Alu.mult)
                    nc.vector.tensor_tensor(out=mean[:, :], in0=mean[:, :],
                                            in1=z1[:, :], op=Alu.add)
                    nc.scalar.activation(mean[:, :], mean[:, :], Act.Abs)
                    nc.vector.tensor_scalar(out=mean[:, :], in0=mean[:, :],
                                            scalar1=EPS, scalar2=None, op0=Alu.max)
                    nc.vector.reciprocal(tbl_sb[:, c * 8:(c + 1) * 8], mean[:, :])
                # bounce: sbuf [8hi,(c,lo)] -> dram [c, hi*8+lo] -> bcast [P, 2G]
                nc.sync.dma_start(
                    out=scratch.ap().rearrange("c (h l) -> h c l", h=8),
                    in_=tbl_sb[:, :].rearrange("h (c l) -> h c l", c=2))
                nc.sync.dma_start(
                    out=tblrep[:, :],
                    in_=scratch.ap().rearrange("c g -> (c g)").unsqueeze(0)
                        .broadcast_to([P, 2 * G]))

            # ================= Phase B: streaming losses =================
            with (
                tc.tile_pool(name="st3", bufs=2) as st3,
                tc.tile_pool(name="st1", bufs=2) as st1,
                tc.tile_pool(name="gsc", bufs=1) as gsc,
                tc.tile_pool(name="sc3", bufs=1) as sc3,
                tc.tile_pool(name="sc1", bufs=1) as sc1,
                tc.tile_pool(name="red", bufs=1) as redp,
            ):
                for i in range(NT):
                    fs = slice(i * FA, (i + 1) * FA)
                    fs3 = slice(i * FA * 3, (i + 1) * FA * 3)
                    w_b3 = w_res[:, fs].unsqueeze(2).broadcast_to([P, FA, 3])
                    w_b2 = w_res[:, fs].unsqueeze(2).broadcast_to([P, FA, 2])

                    def accum(col, part):
                        nc.vector.tensor_tensor(out=stats_t[:, col:col + 1],
                                                in0=stats_t[:, col:col + 1],
                                                in1=part[:, 0:1], op=Alu.add)

                    # ---- gather: rp/rt [P, FA] ----
                    rp_t = gsc.tile([P, FA], dt.float32, tag="rp")
                    rt_t = gsc.tile([P, FA], dt.float32, tag="rt")
                    for j in range(FA // FG):
                        js = slice(i * FA + j * FG, i * FA + (j + 1) * FG)
                        jo = slice(j * FG, (j + 1) * FG)
                        oh = gsc.tile([P, FG * G], dt.float32, tag="oh")
                        prod = gsc.tile([P, FG * G], dt.float32, tag="prod")
                        ohr = oh[:, :].rearrange("p (f g) -> p f g", g=G)
                        prodr = prod[:, :].rearrange("p (f g) -> p f g", g=G)
                        gm_b = gmx_res[:, js].unsqueeze(2).broadcast_to([P, FG, G])
                        i64_b = iota64.unsqueeze(1).broadcast_to([P, FG, G])
                        nc.vector.tensor_tensor(out=ohr, in0=gm_b, in1=i64_b,
                                                op=Alu.is_equal)
                        for c, dst in ((0, rp_t), (1, rt_t)):
                            tb = tblrep[:, c * G:(c + 1) * G].unsqueeze(1) \
                                .broadcast_to([P, FG, G])
                            nc.vector.tensor_tensor(out=prodr, in0=ohr, in1=tb,
                                                    op=Alu.mult)
                            nc.vector.tensor_reduce(out=dst[:, jo], in_=prodr,
                                                    axis=AX, op=Alu.add)

                    # ---- l_3d ----
                    rp_b = rp_t[:, :].unsqueeze(2).broadcast_to([P, FA, 3])
                    rt_b = rt_t[:, :].unsqueeze(2).broadcast_to([P, FA, 3])
                    Pv = P_res[:, :].rearrange("p (f c) -> p f c", c=3)
                    Tv = T_res[:, :].rearrange("p (f c) -> p f c", c=3)
                    qp = sc3.tile([P, FA * 3], dt.float32, tag="qp")
                    qt = sc3.tile([P, FA * 3], dt.float32, tag="qt")
                    qp3 = qp[:, :].rearrange("p (f c) -> p f c", c=3)
                    qt3 = qt[:, :].rearrange("p (f c) -> p f c", c=3)
                    nc.vector.tensor_tensor(out=qp3, in0=Pv[:, fs, :], in1=rp_b,
                                            op=Alu.mult)
                    nc.vector.tensor_tensor(out=qt3, in0=Tv[:, fs, :], in1=rt_b,
                                            op=Alu.mult)
                    # qp <- ln(1+|qp|), qt <- ln(1+|qt|) (in-place ACT)
                    nc.scalar.activation(qp[:, :], qp[:, :], Act.Abs)
                    nc.scalar.activation(qp[:, :], qp[:, :], Act.Ln, bias=1.0)
                    nc.scalar.activation(qt[:, :], qt[:, :], Act.Abs)
                    nc.scalar.activation(qt[:, :], qt[:, :], Act.Ln, bias=1.0)
                    sg = sc3.tile([P, FA * 3], dt.float32, tag="sg")
                    nc.vector.tensor_tensor(out=sg[:, :], in0=P_res[:, fs3],
                                            in1=T_res[:, fs3], op=Alu.mult)
                    nc.scalar.activation(sg[:, :], sg[:, :], Act.Sign)
                    nc.vector.tensor_tensor(out=sg[:, :], in0=sg[:, :], in1=qt[:, :],
                                            op=Alu.mult)
                    nc.vector.tensor_tensor(out=sg[:, :], in0=qp[:, :], in1=sg[:, :],
                                            op=Alu.subtract)
                    sg3 = sg[:, :].rearrange("p (f c) -> p f c", c=3)
                    nc.vector.tensor_tensor(out=sg3, in0=sg3, in1=w_b3, op=Alu.mult)
                    part = redp.tile([P, 1], dt.float32, tag="part")
                    nc.vector.tensor_reduce(out=part[:, :], in_=sg[:, :], axis=AX,
                                            op=Alu.add, apply_absolute_value=True)
                    accum(0, part)

                    # ---- l_2d ----
                    a2 = st1.tile([P, FA * 2], dt.float32, tag="a2")
                    b2 = st1.tile([P, FA * 2], dt.float32, tag="b2")
                    nc.sync.dma_start(out=a2[:, :], in_=v3t(p2, 2, i))
                    nc.sync.dma_start(out=b2[:, :], in_=v3t(t2, 2, i))
                    nc.vector.tensor_tensor(out=a2[:, :], in0=a2[:, :], in1=b2[:, :],
                                            op=Alu.subtract)
                    a23 = a2[:, :].rearrange("p (f c) -> p f c", c=2)
                    nc.vector.tensor_tensor(out=a23, in0=a23, in1=w_b2, op=Alu.mult)
                    part = redp.tile([P, 1], dt.float32, tag="part")
                    nc.vector.tensor_reduce(out=part[:, :], in_=a2[:, :], axis=AX,
                                            op=Alu.add, apply_absolute_value=True)
                    accum(1, part)

                    # ---- l_vis (BCE) ----
                    vv = st1.tile([P, FA * 2], dt.float32, tag="vv")
                    xv = vv[:, 0:FA]
                    tvv = vv[:, FA:2 * FA]
                    nc.sync.dma_start(out=xv, in_=v3t(pv, 1, i))
                    nc.sync.dma_start(out=tvv, in_=v1t(tv, i))
                    xt = sc1.tile([P, FA], dt.float32, tag="xt")
                    nc.vector.tensor_tensor(out=xt[:, :], in0=xv, in1=tvv,
                                            op=Alu.mult)
                    bmax = sc1.tile([P, FA], dt.float32, tag="bmax")
                    nc.vector.scalar_tensor_tensor(out=bmax[:, :], in0=xv,
                                                   scalar=0.0, in1=xt[:, :],
                                                   op0=Alu.max, op1=Alu.subtract)
                    sp_t = sc1.tile([P, FA], dt.float32, tag="sp")
                    nc.scalar.activation(sp_t[:, :], xv, Act.Abs)
                    nc.scalar.activation(sp_t[:, :], sp_t[:, :], Act.Exp, scale=-1.0)
                    nc.scalar.activation(sp_t[:, :], sp_t[:, :], Act.Ln, bias=1.0)
                    nc.vector.tensor_tensor(out=sp_t[:, :], in0=sp_t[:, :],
                                            in1=bmax[:, :], op=Alu.add)
                    nc.vector.tensor_tensor(out=sp_t[:, :], in0=sp_t[:, :],
                                            in1=w_res[:, fs], op=Alu.mult)
                    part = redp.tile([P, 1], dt.float32, tag="part")
                    nc.vector.tensor_reduce(out=part[:, :], in_=sp_t[:, :], axis=AX,
                                            op=Alu.add)
                    accum(2, part)

                    # ---- l_disp ----
                    a3 = st3.tile([P, FA * 3], dt.float32, tag="a3")
                    b3 = st3.tile([P, FA * 3], dt.float32, tag="b3")
                    nc.sync.dma_start(out=a3[:, :], in_=v3t(pd, 3, i))
                    nc.sync.dma_start(out=b3[:, :], in_=v3t(td, 3, i))
                    nc.vector.tensor_tensor(out=a3[:, :], in0=a3[:, :], in1=b3[:, :],
                                            op=Alu.subtract)
                    a33 = a3[:, :].rearrange("p (f c) -> p f c", c=3)
                    nc.vector.tensor_tensor(out=a33, in0=a33, in1=w_b3, op=Alu.mult)
                    part = redp.tile([P, 1], dt.float32, tag="part")
                    nc.vector.tensor_reduce(out=part[:, :], in_=a3[:, :], axis=AX,
                                            op=Alu.add, apply_absolute_value=True)
                    accum(3, part)

                    # ---- l_normal: accumulate sum(w * cos) ----
                    n3 = st3.tile([P, FA * 3], dt.float32, tag="a3")
                    m3 = st3.tile([P, FA * 3], dt.float32, tag="b3")
                    nc.sync.dma_start(out=n3[:, :], in_=v3t(pn, 3, i))
                    nc.sync.dma_start(out=m3[:, :], in_=v3t(tn, 3, i))
                    n33 = n3[:, :].rearrange("p (f c) -> p f c", c=3)
                    m33 = m3[:, :].rearrange("p (f c) -> p f c", c=3)
                    pr = sc3.tile([P, FA * 3], dt.float32, tag="sg")
                    pr3 = pr[:, :].rearrange("p (f c) -> p f c", c=3)
                    ppn = sc1.tile([P, FA], dt.float32, tag="xt")
                    ttn = sc1.tile([P, FA], dt.float32, tag="bmax")
                    dotn = sc1.tile([P, FA], dt.float32, tag="sp")
                    nc.vector.tensor_tensor(out=pr3, in0=n33, in1=n33, op=Alu.mult)
                    nc.vector.tensor_reduce(out=ppn[:, :], in_=pr3, axis=AX,
                                            op=Alu.add)
                    nc.vector.tensor_tensor(out=pr3, in0=m33, in1=m33, op=Alu.mult)
                    nc.vector.tensor_reduce(out=ttn[:, :], in_=pr3, axis=AX,
                                            op=Alu.add)
                    nc.vector.tensor_tensor(out=pr3, in0=n33, in1=m33, op=Alu.mult)
                    nc.vector.tensor_reduce(out=dotn[:, :], in_=pr3, axis=AX,
                                            op=Alu.add)
                    nc.vector.tensor_tensor(out=ppn[:, :], in0=ppn[:, :],
                                            in1=ttn[:, :], op=Alu.mult)
                    # rsqrt(u) = exp(-0.5*ln(u))
                    nc.scalar.activation(ppn[:, :], ppn[:, :], Act.Ln)
                    nc.scalar.activation(ppn[:, :], ppn[:, :], Act.Exp, scale=-0.5)
                    nc.vector.tensor_tensor(out=dotn[:, :], in0=dotn[:, :],
                                            in1=ppn[:, :], op=Alu.mult)
                    nc.vector.tensor_tensor(out=dotn[:, :], in0=dotn[:, :],
                                            in1=w_res[:, fs], op=Alu.mult)
                    part = redp.tile([P, 1], dt.float32, tag="part")
                    nc.vector.tensor_reduce(out=part[:, :], in_=dotn[:, :], axis=AX,
                                            op=Alu.add)
                    accum(4, part)

                    # ---- l_conf ----
                    cfv = st1.tile([P, FA], dt.float32, tag="cfv")
                    nc.sync.dma_start(out=cfv[:, :], in_=v3t(cf, 1, i))
                    nc.vector.tensor_tensor(out=cfv[:, :], in0=cfv[:, :],
                                            in1=w_res[:, fs], op=Alu.mult)
                    part = redp.tile([P, 1], dt.float32, tag="part")
                    nc.vector.tensor_reduce(out=part[:, :], in_=cfv[:, :], axis=AX,
                                            op=Alu.add)
                    accum(5, part)

            nc.sync.dma_start(out=stats_out[:, :], in_=stats_t[:, :])

    nc.compile()
    return nc


def kernel(**inputs):
    nc = _COMPILED.get("nc")
    if nc is None:
        nc = _build()
        _COMPILED["nc"] = nc

    in_maps = [{k: np.ascontiguousarray(v[b]) for k, v in inputs.items()}
               for b in range(B)]
    res = run_bass_kernel_spmd(nc, in_maps, core_ids=list(range(8)))

    tot = dict(s3d=0.0, s2d=0.0, svis=0.0, sdisp=0.0, snorm=0.0, sconf=0.0, cnt=0.0)
    for b in range(B):
        r = res.results[b]
        g = r["gstats"].astype(np.float64)
        s = r["stats"].astype(np.float64).sum(axis=0)
        cnt_b = g[:, 0:8].sum()
        tot["cnt"] += cnt_b
        tot["s3d"] += s[0]
        tot["s2d"] += s[1]
        tot["svis"] += s[2]
        tot["sdisp"] += s[3]
        tot["snorm"] += cnt_b - s[4]
        tot["sconf"] += s[5]

    V = tot["cnt"]
    loss = (1.0 * tot["s3d"] / (3 * V + 1e-6)
            + 0.1 * tot["s2d"] / (2 * V + 1e-6)
            + 0.1 * tot["svis"] / (V + 1e-6)
            + 0.1 * tot["sdisp"] / (3 * V + 1e-6)
            + 0.5 * tot["snorm"] / (V + 1e-6)
            + 0.2 * tot["sconf"] / (V + 1e-6))
    return np.float32(loss)
